# revision 1
# baseline (speedup 1.0000x reference)
"""Trainium2 Bass kernel for CropAndResize (bilinear, TF semantics).

Design (8 NeuronCores, image-sharded; boxes routed by box_ind):

The POOL ap_gather costs ~31 ns PER INDEX regardless of d or channel
count, so the kernel is built around index-count minimization: ONE
gather index per output point. Each index fetches a d=8 block holding
all four bilinear neighbors for TWO channels:

    W8[p, k, :] = (A[k], A[k+1], A[k+W], A[k+W+1],
                   B[k], B[k+1], B[k+W], B[k+W+1])

where partition p carries channel pair (p, p+128) and k = r*160 + x is
a local row/col offset. The 128 KB/partition gather-window limit means
only ~41 image rows fit at 16 B/px, so the image is processed in 4
row-phases (40 rows + 1 halo row each). The host sorts output points
by the row-phase of their top neighbor, pads each phase to fixed-size
gather calls, uploads wrapped int16 indices and premultiplied bilinear
weights (w_tl,w_tr,w_bl,w_br per point; zeros for out-of-range points,
TF extrapolation_value=0), and inverse-permutes the output at the end.

Device loop per phase: rebuild W8 (chunked DMA loads + strided
fp32->fp16 copies split over Scalar/Vector), then per 768-point call:
one ap_gather, one 2x-mode MUL by weights (B reuses A's weights via a
stride-0 AP), two pairwise ADD reductions, and a large-descriptor
store every 2 calls. Host reassembles [point, 256ch] and upcasts.
"""

import numpy as np

import concourse.bass as bass
import concourse.bacc as bacc
import concourse.tile as tile
from concourse import mybir
from concourse.bass_utils import run_bass_kernel_spmd

B, C, H, W = 8, 256, 160, 160
CH, CW = 14, 14
HW = H * W
N_CORES = 8
NPH = 4          # row phases
RPP = 40         # rows per phase
NEB = RPP * W    # gather blocks per phase (6400)
SRCL = (RPP + 1) * W + 1  # local source span: 41 rows + 1 elem (6561)
NI = 768         # points per gather call
NCHK = 4         # W8 build chunks per phase
CKB = NEB // NCHK  # blocks per chunk (1600)

F32 = mybir.dt.float32
F16 = mybir.dt.float16
I16 = mybir.dt.int16

_PROGRAM_CACHE = {}


def _ap(base, extra_offset, pattern):
    return bass.AP(base.tensor, base.offset + extra_offset, pattern)


def build_program(ncalls):
    """ncalls: tuple of per-phase gather-call counts."""
    total_calls = sum(ncalls)
    nstores = (total_calls + 1) // 2
    nc = bacc.Bacc("TRN2", target_bir_lowering=False, debug=False)

    img_d = nc.dram_tensor("img", [257 * HW], F32, kind="ExternalInput")
    idx_d = nc.dram_tensor("idxs", [128, total_calls * (NI // 16)], I16,
                           kind="ExternalInput")
    w4_d = nc.dram_tensor("w4", [total_calls, NI * 4], F16, kind="ExternalInput")
    out_d = nc.dram_tensor("out", [nstores, 128, 2 * NI * 2], F16,
                           kind="ExternalOutput")

    A = mybir.AluOpType
    ADD, MUL = A.add, A.mult

    with tile.TileContext(nc) as tc:
        with (
            tc.tile_pool(name="big", bufs=1) as bigp,
            tc.tile_pool(name="chk", bufs=2) as chkp,
            tc.tile_pool(name="gtp", bufs=2) as gtp,
            tc.tile_pool(name="wp", bufs=2) as wp,
            tc.tile_pool(name="stg", bufs=2) as stgp,
        ):
            idxs = bigp.tile([128, total_calls * (NI // 16)], I16, tag="idxs")
            nc.sync.dma_start(idxs[:], idx_d[:])
            w8 = bigp.tile([128, NEB, 8], F16, tag="w8")

            ci = 0  # global call counter
            stgt = None
            for ph in range(NPH):
                if ncalls[ph] == 0:
                    continue
                # ---- build W8 for this phase's 41-row window ----
                for kc in range(NCHK):
                    # local block range [kc*CKB, (kc+1)*CKB); source needs
                    # [kc*CKB, kc*CKB + CKB + W + 1)
                    for half, src_ch in ((0, 0), (1, 128)):
                        sc = chkp.tile([128, CKB + W + 1], F32, tag="src")
                        off = (src_ch * HW + ph * RPP * W + kc * CKB)
                        nc.sync.dma_start(
                            sc[:], _ap(img_d[:], off, [[HW, 128], [1, CKB + W + 1]])
                        )
                        o = kc * CKB
                        s0 = half * 4
                        for s, so in ((0, 0), (1, 1), (2, W), (3, W + 1)):
                            eng = nc.scalar if (s % 2 == 0) else nc.vector
                            dst = _ap(w8[:], o * 8 + s0 + s,
                                      [w8[:].ap[0], [8, CKB]])
                            src = sc[:, so : so + CKB]
                            if eng is nc.scalar:
                                eng.copy(dst, src)
                            else:
                                eng.tensor_copy(out=dst, in_=src)
                # ---- gather calls for this phase ----
                for _ in range(ncalls[ph]):
                    gt = gtp.tile([128, NI, 8], F16, tag="gt")
                    nc.gpsimd.ap_gather(
                        gt[:], w8[:],
                        idxs[:, ci * (NI // 16) : (ci + 1) * (NI // 16)],
                        channels=128, num_elems=NEB, d=8, num_idxs=NI,
                    )
                    w4 = wp.tile([128, NI * 4], F16, tag="w4")
                    nc.sync.dma_start(
                        w4[:], _ap(w4_d[:], ci * NI * 4, [[0, 128], [1, NI * 4]])
                    )
                    # gt[q, ch, slot] *= w4[q, slot]  (ch broadcast, 2x mode)
                    g_ap = _ap(gt[:], 0, [gt[:].ap[0], [8, NI], [4, 2], [1, 4]])
                    w_ap = _ap(w4[:], 0, [w4[:].ap[0], [4, NI], [0, 2], [1, 4]])
                    nc.vector.tensor_tensor(out=g_ap, in0=g_ap, in1=w_ap, op=MUL)
                    # reduce 4 taps -> 1 per (q, ch)
                    o1 = wp.tile([128, NI * 4], F16, tag="o1")
                    e_v = _ap(gt[:], 0, [gt[:].ap[0], [2, NI * 4]])
                    d_v = _ap(gt[:], 1, [gt[:].ap[0], [2, NI * 4]])
                    nc.vector.tensor_tensor(out=o1[:], in0=e_v, in1=d_v, op=ADD)
                    if ci % 2 == 0:
                        stgt = stgp.tile([128, 2, NI * 2], F16, tag="stg")
                    so = stgt[:, ci % 2, :]
                    e2 = _ap(o1[:], 0, [o1[:].ap[0], [2, NI * 2]])
                    d2 = _ap(o1[:], 1, [o1[:].ap[0], [2, NI * 2]])
                    nc.vector.tensor_tensor(out=so, in0=e2, in1=d2, op=ADD)
                    if ci % 2 == 1:
                        nc.sync.dma_start(
                            out_d[ci // 2],
                            stgt[:].rearrange("p a b -> p (a b)"),
                        )
                    ci += 1
            if ci % 2 == 1:  # flush odd final store
                nc.sync.dma_start(
                    out_d[ci // 2], stgt[:].rearrange("p a b -> p (a b)")
                )

    nc.compile()
    return nc


def _host_points(bk):
    """Per-point phase, local block index, and premultiplied weights."""
    f = np.float32
    iota = np.arange(CH, dtype=f)
    y1, x1, y2, x2 = bk[:, 0], bk[:, 1], bk[:, 2], bk[:, 3]

    def axis(lo, hi):
        scale = (hi - lo) * f(H - 1) / f(CH - 1)
        inv = lo[:, None] * f(H - 1) + iota[None, :] * scale[:, None]
        valid = (inv >= f(0)) & (inv <= f(H - 1))
        fl = np.floor(inv)
        frac = (inv - fl).astype(f)
        lo_i = np.clip(fl, 0, H - 1).astype(np.int64)
        return valid, frac, lo_i

    vy, yl, ti = axis(y1, y2)
    vx, xl, li = axis(x1, x2)

    valid = vy[:, :, None] & vx[:, None, :]          # [nb,14,14]
    ph = np.where(valid, ti[:, :, None] // RPP, 0)
    kloc = (ti[:, :, None] - ph * RPP) * W + li[:, None, :]
    kloc = np.where(valid, kloc, 0)
    wy1 = yl[:, :, None]
    wx1 = xl[:, None, :]
    w4p = np.stack(
        [(1 - wy1) * (1 - wx1), (1 - wy1) * wx1, wy1 * (1 - wx1), wy1 * wx1],
        axis=-1,
    )
    w4p = np.where(valid[..., None], w4p, f(0))
    return (
        ph.reshape(-1),
        kloc.reshape(-1).astype(np.int16),
        w4p.reshape(-1, 4).astype(np.float16),
    )


def _host_streams(phf, kf, wf, ncalls):
    """Build phase-sorted padded streams for one core given common ncalls."""
    npts = phf.shape[0]
    idx_stream, w_stream, pt_stream = [], [], []
    for p in range(NPH):
        if ncalls[p] == 0:
            continue
        sel = np.nonzero(phf == p)[0]
        pad = ncalls[p] * NI - len(sel)
        idx_stream.append(np.concatenate([kf[sel], np.zeros(pad, np.int16)]))
        w_stream.append(np.concatenate([wf[sel], np.zeros((pad, 4), np.float16)]))
        pt_stream.append(np.concatenate([sel, np.full(pad, -1, np.int64)]))
    tc = sum(ncalls)
    idx_flat = np.concatenate(idx_stream)          # [tc*NI]
    w4 = np.concatenate(w_stream).reshape(tc, NI * 4)
    pt_of_stream = np.concatenate(pt_stream)

    # wrap: position q of call c reads idx tile row q%16, slot (c*(NI//16) + q//16)
    wrap = idx_flat.reshape(tc, NI // 16, 16).transpose(2, 0, 1).reshape(
        16, tc * (NI // 16)
    )
    idxs = np.tile(wrap, (8, 1))                   # [128, tc*NI//16]
    return idxs, w4, pt_of_stream, npts


def make_in_maps(image, boxes, box_ind):
    image = np.asarray(image, dtype=np.float32)
    boxes = np.asarray(boxes, dtype=np.float32)
    box_ind = np.asarray(box_ind, dtype=np.int32)

    order = np.argsort(box_ind, kind="stable")
    counts = np.bincount(box_ind, minlength=N_CORES)
    starts = np.zeros(N_CORES + 1, np.int64)
    starts[1:] = np.cumsum(counts)
    cap = max(1, int(counts.max()))

    pts = []
    for k in range(N_CORES):
        bk = np.zeros((cap, 4), np.float32)
        sel = order[starts[k] : starts[k + 1]]
        bk[: counts[k]] = boxes[sel]
        pts.append(_host_points(bk))
    # common per-phase call counts across cores (one SPMD program)
    ncalls = tuple(
        int(max(-(-np.count_nonzero(p[0] == ph) // NI) for p in pts))
        for ph in range(NPH)
    )

    in_maps, metas = [], []
    for k in range(N_CORES):
        img_k = np.empty(257 * HW, np.float32)
        img_k[: 256 * HW] = image[k].reshape(-1)
        img_k[256 * HW :] = 0.0
        idxs, w4, pt_of_stream, npts = _host_streams(*pts[k], ncalls)
        in_maps.append({"img": img_k, "idxs": idxs, "w4": w4})
        metas.append((pt_of_stream, ncalls, npts))
    return in_maps, order, counts, starts, cap, metas


def kernel(image, boxes, box_ind):
    in_maps, order, counts, starts, cap, metas = make_in_maps(
        image, boxes, box_ind
    )
    key = metas[0][1]
    nc = _PROGRAM_CACHE.get(key)
    if nc is None:
        nc = build_program(key)
        _PROGRAM_CACHE[key] = nc

    res = run_bass_kernel_spmd(nc, in_maps, core_ids=list(range(N_CORES)))

    n = boxes.shape[0]
    out = np.empty((n, C, CH, CW), np.float32)
    for k in range(N_CORES):
        pt_of_stream, ncalls, npts = metas[k]
        sel = order[starts[k] : starts[k + 1]]
        arr = res.results[k]["out"]  # [nstores, 128, 2*NI*2] f16
        tcall = sum(ncalls)
        arr = arr.reshape(-1, 128, 2, NI, 2)[:, :, :, :, :]
        # [store, p, sub, q, chbit] -> stream index s = (store*2+sub)*NI + q
        arr = arr.transpose(0, 2, 3, 4, 1).reshape(-1, 2, 128)
        arr = arr.reshape(-1, 256)[: tcall * NI]  # [s, chbit*128+p]
        keep = pt_of_stream >= 0
        pts = np.empty((npts, 256), np.float16)
        pts[pt_of_stream[keep]] = arr[keep]
        ok = (
            pts.reshape(cap, CH, CW, 256)
            .transpose(0, 3, 1, 2)
            .astype(np.float32)[: counts[k]]
        )
        out[sel] = ok
    return out



# revision 7
# speedup vs baseline: 1.8495x; 1.8495x over previous
"""Trainium2 Bass kernel for CropAndResize (bilinear, TF semantics).

Design (8 NeuronCores, image batch-sharded; boxes routed by box_ind):

Each core handles one image of the batch and the boxes pointing at it
(padded to the max per-core box count so one SPMD program serves all
cores). Partition p carries channel pair (p, p+128).

The host casts the image to f16. The device processes the image in 4
row-phases (40 rows + 1 halo row). Per phase a gather space holds, for
each channel half, an even-pair and an odd-pair copy of the window:

    seg[half][par][m] = (w[2m+par], w[2m+par+1])     (d=2 f16 blocks)

built by 4 plain contiguous DMA loads (byte offsets 0 / +1 element) —
no compute-engine table build. Output points are sorted by the phase of
their top row; 4 gather indices per point fetch the (top,left/right)
and (bottom,left/right) pairs for both channel halves in ONE ap_gather
per 768-point call. Two f16 MULs apply host-premultiplied bilinear
weights (zeroed for out-of-range taps, TF extrapolation=0), one
tensor_reduce sums the 4 taps, and results are stored 2 calls per DMA.

Per-core call counts are equalized across cores via prefix-cum-max
padding at phase boundaries; two window slots form a ring so window
loads overlap compute. The host inverse-permutes the output.
"""

import numpy as np

import concourse.bass as bass
import concourse.bacc as bacc
import concourse.tile as tile
from concourse import mybir
from concourse.bass_utils import run_bass_kernel_spmd

B, C, H, W = 8, 256, 160, 160
CH, CW = 14, 14
HW = H * W
N_CORES = 8
NPH = 4          # row phases
RPP = 40         # rows per phase
WIN = (RPP + 1) * W          # window elements per channel half (6560)
NPAIR = WIN // 2             # pairs per parity segment (3280)
NELEM = 2 * 2 * NPAIR        # gather num_elems per window (13120)
IMGP = NPH * RPP * W + WIN + 4   # padded image length per channel (25764)
NI = 768         # points per gather call
IPP = 4          # gather indices per point

F32 = mybir.dt.float32
F16 = mybir.dt.float16
I16 = mybir.dt.int16

_PROGRAM_CACHE = {}


def _ap(base, extra_offset, pattern):
    return bass.AP(base.tensor, base.offset + extra_offset, pattern)


def build_program(key):
    """key: (total_calls, E0, E1) — E0/E1 are call indices at which the
    window-2 / window-3 ring reloads are issued (all reads of window 0 /
    window 1 happen in calls before E0 / E1 respectively)."""
    total_calls, E0, E1 = key
    nstores = (total_calls + 1) // 2
    nc = bacc.Bacc("TRN2", target_bir_lowering=False, debug=False)

    img_d = nc.dram_tensor("img", [256 * IMGP], F16, kind="ExternalInput")
    idx_d = nc.dram_tensor("idxs", [128, total_calls * (IPP * NI // 16)], I16,
                           kind="ExternalInput")
    w4_d = nc.dram_tensor("w4", [total_calls, NI * 4], F16, kind="ExternalInput")
    out_d = nc.dram_tensor("out", [nstores, 128, 2 * NI * 2], F16,
                           kind="ExternalOutput")

    A = mybir.AluOpType
    ADD, MUL = A.add, A.mult

    with tile.TileContext(nc) as tc:
        with (
            tc.tile_pool(name="big", bufs=1) as bigp,
            tc.tile_pool(name="gtp", bufs=3) as gtp,
            tc.tile_pool(name="wp", bufs=3) as wp,
            tc.tile_pool(name="stg", bufs=2) as stgp,
        ):
            idxs = bigp.tile([128, total_calls * (IPP * NI // 16)], I16,
                             tag="idxs")
            nc.sync.dma_start(idxs[:], idx_d[:])
            # gather ring: 2 window slots x (half, parity) x WIN f16
            G = bigp.tile([128, 2, 2, 2, WIN], F16, tag="G")

            def load_window(ph):
                slot = ph % 2
                for half in (0, 1):
                    for par in (0, 1):
                        off = half * 128 * IMGP + ph * RPP * W + par
                        nc.sync.dma_start(
                            G[:, slot, half, par, :],
                            _ap(img_d[:], off, [[IMGP, 128], [1, WIN]]),
                        )

            load_window(0)
            load_window(1)
            # gather space: both ring slots (calls may straddle a phase
            # boundary; the host guarantees every index in call ci targets
            # a window resident at ci given the reload points E0/E1)
            in_ap = _ap(G[:], 0, [G[:].ap[0], [2, 2 * NELEM], [1, 2]])

            stgt = None
            for ci in range(total_calls):
                if ci == E0:
                    load_window(2)
                if ci == E1:
                    load_window(3)
                gt = gtp.tile([128, NI, 8], F16, tag="gt")
                out_ap = _ap(gt[:], 0, [gt[:].ap[0], [2, IPP * NI], [1, 2]])
                nc.gpsimd.ap_gather(
                    out_ap, in_ap,
                    idxs[:, ci * (IPP * NI // 16):(ci + 1) * (IPP * NI // 16)],
                    channels=128, num_elems=2 * NELEM, d=2, num_idxs=IPP * NI,
                )
                w4 = wp.tile([128, NI * 4], F16, tag="w4")
                nc.sync.dma_start(
                    w4[:], _ap(w4_d[:], ci * NI * 4, [[0, 128], [1, NI * 4]])
                )
                # gt layout per point q: [half, tb, e] (8 f16)
                w_ap = _ap(w4[:], 0, [w4[:].ap[0], [4, NI], [1, 4]])
                for half in (0, 1):
                    g_ap = _ap(gt[:], half * 4,
                               [gt[:].ap[0], [8, NI], [1, 4]])
                    nc.vector.tensor_tensor(
                        out=g_ap, in0=g_ap, in1=w_ap, op=MUL
                    )
                if ci % 2 == 0:
                    stgt = stgp.tile([128, 2, NI * 2], F16, tag="stg")
                so = _ap(stgt[:], (ci % 2) * NI * 2,
                         [stgt[:].ap[0], [2, NI], [1, 2]])
                red_in = _ap(gt[:], 0, [gt[:].ap[0], [8, NI], [4, 2], [1, 4]])
                with nc.allow_low_precision(reason="f16 4-tap sum"):
                    nc.vector.tensor_reduce(
                        out=so, in_=red_in, axis=mybir.AxisListType.X, op=ADD
                    )
                if ci % 2 == 1:
                    nc.sync.dma_start(
                        out_d[ci // 2],
                        stgt[:].rearrange("p a b -> p (a b)"),
                    )
            if total_calls % 2 == 1:  # flush odd final store
                nc.sync.dma_start(
                    out_d[total_calls // 2],
                    stgt[:].rearrange("p a b -> p (a b)"),
                )

    nc.compile()
    return nc


def _host_points(bk):
    """Per-point phase, gather base index, and premultiplied weights."""
    f = np.float32
    iota = np.arange(CH, dtype=f)
    y1, x1, y2, x2 = bk[:, 0], bk[:, 1], bk[:, 2], bk[:, 3]

    def axis(lo, hi):
        scale = (hi - lo) * f(H - 1) / f(CH - 1)
        inv = lo[:, None] * f(H - 1) + iota[None, :] * scale[:, None]
        valid = (inv >= f(0)) & (inv <= f(H - 1))
        fl = np.floor(inv)
        frac = (inv - fl).astype(f)
        lo_i = np.clip(fl, 0, H - 1).astype(np.int64)
        return valid, frac, lo_i

    vy, yl, ti = axis(y1, y2)
    vx, xl, li = axis(x1, x2)

    valid = vy[:, :, None] & vx[:, None, :]          # [nb,14,14]
    ti3 = ti[:, :, None]
    ph = np.where(valid, np.minimum(ti3 // RPP, NPH - 1), 0)
    k = (ti3 - ph * RPP) * W + li[:, None, :]
    k = np.where(valid, k, 0)
    wy1 = yl[:, :, None] + 0 * xl[:, None, :]
    wx1 = xl[:, None, :] + 0 * yl[:, :, None]
    w4p = np.stack(
        [(1 - wy1) * (1 - wx1), (1 - wy1) * wx1, wy1 * (1 - wx1), wy1 * wx1],
        axis=-1,
    )
    w4p = np.where(valid[..., None], w4p, f(0))
    return (
        ph.reshape(-1),
        k.reshape(-1).astype(np.int64),
        w4p.reshape(-1, 4).astype(np.float16),
    )


def _idx4(k, ph):
    """4 gather indices (A-top, A-bot, B-top, B-bot) for base offsets k
    in phase ph (ring slot ph%2)."""
    base = (ph % 2) * NELEM

    def pair(kk):
        return (kk & 1) * NPAIR + (kk >> 1)
    top = base + pair(k)
    bot = top + W // 2
    off_b = 2 * NPAIR
    return np.stack([top, bot, top + off_b, bot + off_b], axis=-1)


def _host_streams(phf, kf, wf, ncalls):
    """Phase-sorted padded idx/weight streams for one core."""
    idx_stream, w_stream, pt_stream = [], [], []
    for p in range(NPH):
        if ncalls[p] == 0:
            continue
        sel = np.nonzero(phf == p)[0]
        pad = ncalls[p] * NI - len(sel)
        assert pad >= 0
        kp = np.concatenate([kf[sel], np.zeros(pad, np.int64)])
        idx_stream.append(_idx4(kp, p).reshape(-1).astype(np.int16))
        w_stream.append(np.concatenate([wf[sel], np.zeros((pad, 4), np.float16)]))
        pt_stream.append(np.concatenate([sel, np.full(pad, -1, np.int64)]))
    tc = sum(ncalls)
    idx_flat = np.concatenate(idx_stream)          # [tc*NI*IPP]
    w4 = np.concatenate(w_stream).reshape(tc, NI * 4)
    pt_of_stream = np.concatenate(pt_stream)

    # wrap: idx j of call c -> partition j%16, col c*(IPP*NI//16) + j//16
    npc = IPP * NI // 16
    wrap = idx_flat.reshape(tc, npc, 16).transpose(2, 0, 1).reshape(16, tc * npc)
    idxs = np.tile(wrap, (8, 1))                   # [128, tc*npc]
    return idxs, w4, pt_of_stream, phf.shape[0]


def make_in_maps(image, boxes, box_ind):
    image = np.asarray(image)
    boxes = np.asarray(boxes, dtype=np.float32)
    box_ind = np.asarray(box_ind, dtype=np.int32)

    order = np.argsort(box_ind, kind="stable")
    counts = np.bincount(box_ind, minlength=N_CORES)
    starts = np.zeros(N_CORES + 1, np.int64)
    starts[1:] = np.cumsum(counts)
    cap = max(1, int(counts.max()))

    pts = []
    for k in range(N_CORES):
        bk = np.zeros((cap, 4), np.float32)
        sel = order[starts[k]: starts[k + 1]]
        bk[: counts[k]] = boxes[sel]
        pts.append(_host_points(bk))
    # per-phase-max call counts (common across cores; one SPMD program)
    ncalls = tuple(
        int(max(-(-np.count_nonzero(p[0] == ph) // NI) for p in pts))
        for ph in range(NPH)
    )
    key = (sum(ncalls), ncalls[0], ncalls[0] + ncalls[1])

    in_maps, metas = [], []
    for k in range(N_CORES):
        img_k = np.zeros((256, IMGP), np.float16)
        img_k[:, :HW] = np.asarray(image[k], np.float16).reshape(256, HW)
        idxs, w4, pt_of_stream, npts = _host_streams(*pts[k], ncalls)
        in_maps.append({"img": img_k.reshape(-1), "idxs": idxs, "w4": w4})
        metas.append((pt_of_stream, key, npts))
    return in_maps, order, counts, starts, cap, metas


def kernel(image, boxes, box_ind):
    in_maps, order, counts, starts, cap, metas = make_in_maps(
        image, boxes, box_ind
    )
    key = metas[0][1]
    nc = _PROGRAM_CACHE.get(key)
    if nc is None:
        nc = build_program(key)
        _PROGRAM_CACHE[key] = nc

    res = run_bass_kernel_spmd(nc, in_maps, core_ids=list(range(N_CORES)))

    n = boxes.shape[0]
    out = np.empty((n, C, CH, CW), np.float32)
    for k in range(N_CORES):
        pt_of_stream, key_k, npts = metas[k]
        sel = order[starts[k]: starts[k + 1]]
        arr = res.results[k]["out"]  # [nstores, 128, 2*NI*2] f16
        tcall = key_k[0]
        arr = arr.reshape(-1, 128, 2, NI, 2)
        # [store, p, sub, q, chbit] -> stream index s = (store*2+sub)*NI + q
        arr = arr.transpose(0, 2, 3, 4, 1).reshape(-1, 2, 128)
        arr = arr.reshape(-1, 256)[: tcall * NI]  # [s, chbit*128+p]
        keep = pt_of_stream >= 0
        ptsv = np.empty((npts, 256), np.float16)
        ptsv[pt_of_stream[keep]] = arr[keep]
        ok = (
            ptsv.reshape(cap, CH, CW, 256)
            .transpose(0, 3, 1, 2)
            .astype(np.float32)[: counts[k]]
        )
        out[sel] = ok
    return out
